# revision 15
# baseline (speedup 1.0000x reference)
"""Trainium2 Bass kernel for nn_BiDisen (gnn_message_passing).

Self-contained: takes FULL inputs, shards across 8 NeuronCores, runs one SPMD
Bass/Tile program, gathers FULL outputs (user_emb, item_emb, loss, pr).

Math (validated vs reference):
  routing_conv: softmax over neighbors of e_ij = x_i.att1 + z_j.att2 — the x_i
  term cancels in softmax, so attn_ij = s_j / sum_m s_j with s_j = exp(z_j.att2).
  Layer output: u_i = x_i + a * (sum_m y_{j(i,m)}) / (sum_m s_{j(i,m)}) where
  y_j = s_j * t_jk * z_j (per 16-elem segment k). So each layer = per-node table
  build + unweighted 20-neighbor gather-sums (dma_gather + ap_gather).

  KL(emb, T): with y = Mm/sigma (row-normalized), ey = exp(y), v = ey-1,
  t = tanh(S/2) (sigmoid(S) = (1+t)/2), per row:
    kl_i = N/SM + 1/2 + ln(SXr'/SM),  SM = 6000 + sum(v),
    N = 1 + rsig*sum(v*Mm) - (sum(t) + sum(v*t))/2 - SM/2,
    SXr' = sum(exp(t/2)) - n_pad_cols.
  All f32 (the KL signal lives at 1e-4 scale — bf16 destroys it).

Node order: tau'-space — core c owns rows [c*1536, (c+1)*1536) laid out as
[user 750 | pad 18 | item 750 | pad 18]; all index inputs are pre-mapped on the
host into tau'-space (and into the s-blob address spaces for ap_gather).
"""

import sys

for _p in ("/opt/trn_rl_repo",):
    if _p not in sys.path:
        sys.path.insert(0, _p)

import numpy as np

P = 128
D = 128
NB = 20
K8 = 8
NCORE = 8
MLEN = 6000
BLK = 750          # rows per core per matrix (user / item)
SPAD = 768         # padded block (6*128)
SL = 1536          # own combined slice (user block + item block)
NT = 12            # node tiles per core
NBT = 96           # build tiles for the full table (12288/128)
TAB = 12288        # table rows (8*1536, tau'-space)
UW = 6144          # padded width of one KL matrix (8*768)
NSTRIP = 6         # row strips per matrix per core
NCH = 12           # 512-col chunks per strip
CHW = 512
RTOT = 100000
RC = RTOT // NCORE         # 12500 pairs per core
RPAD = 12800               # 100*128
NPRT = RPAD // P           # 100
PRCH = 4                   # pr gather chunks
PRCI = RPAD // PRCH        # 3200 idxs per chunk
PRCT = PRCI // P           # 25 tiles per chunk
AA = 0.9
NPADC = 144.0              # pad cols per KL row (8*18): exp(tanh(0)/2)=1 each
TW = 192                   # fused table row: [y(128) | s | 63 zeros], 768B

F32 = None  # set lazily (mybir.dt.float32)


# ----------------------------------------------------------------------------
# host-side index/shard prep (pure numpy data movement + index arithmetic)
# ----------------------------------------------------------------------------

def _tau(j):
    """combined node id (0..11999) -> tau' table row (0..12287)."""
    j = np.asarray(j, np.int64)
    is_item = j >= MLEN
    jj = np.where(is_item, j - MLEN, j)
    c = jj // BLK
    loc = jj % BLK
    return (c * SL + np.where(is_item, SPAD, 0) + loc).astype(np.int64)


def _wrap16(logical):
    """logical idx list (n, n%16==0) -> [128, n//16] int16 in dma_gather wrap
    order: idx[q, s] = logical[s*16 + q%16], replicated across the 8 cores."""
    a = np.asarray(logical, np.int16).reshape(-1, 16).T  # [16, n/16]
    return np.tile(a, (8, 1))


def _prep(inputs):
    neiberm = np.asarray(inputs["neiberm"], np.int64)
    neibern = np.asarray(inputs["neibern"], np.int64)
    duser = np.asarray(inputs["duser"], np.int64)
    ditemid = np.asarray(inputs["ditemid"], np.int64)
    u0 = np.asarray(inputs["u0"], np.float32)
    v0 = np.asarray(inputs["v0"], np.float32)
    att = np.asarray(inputs["att"], np.float32)
    Mm = np.asarray(inputs["Mm"], np.float32)
    Nm = np.asarray(inputs["Nm"], np.float32)

    nbs = np.concatenate([neiberm, neibern + MLEN], 0)        # [12000, 20]
    nbs_tau = _tau(nbs)                                        # [12000, 20]

    # x0 in tau' layout, zero pads
    x0_full = np.zeros((TAB, D), np.float32)
    for c in range(NCORE):
        x0_full[c * SL: c * SL + BLK] = u0[c * BLK:(c + 1) * BLK]
        x0_full[c * SL + SPAD: c * SL + SPAD + BLK] = v0[c * BLK:(c + 1) * BLK]

    in_maps = []
    for c in range(NCORE):
        # own combined node ids per local slot (pads -> user row c*BLK, idx 0 used)
        own = np.zeros(SL, np.int64)
        own_valid = np.zeros(SL, bool)
        own[:BLK] = np.arange(c * BLK, (c + 1) * BLK)
        own_valid[:BLK] = True
        own[SPAD:SPAD + BLK] = MLEN + np.arange(c * BLK, (c + 1) * BLK)
        own_valid[SPAD:SPAD + BLK] = True

        nbt_own = np.zeros((SL, NB), np.int64)                 # tau' of neighbors
        nbt_own[own_valid] = nbs_tau[own[own_valid]]

        idx_g = np.zeros((NT, P, NB * P // 16), np.int16)
        for t in range(NT):
            blk = nbt_own[t * P:(t + 1) * P]                   # [128, 20]
            idx_g[t] = _wrap16(blk.T.reshape(-1))              # logical n=m*128+p

        pru = np.zeros(RPAD, np.int64)
        pru[:RC] = _tau(duser[c * RC:(c + 1) * RC])
        pri = np.zeros(RPAD, np.int64)
        pri[:RC] = _tau(MLEN + ditemid[c * RC:(c + 1) * RC])

        def padmat(M):
            ms = np.zeros((SPAD, UW), np.float32)
            rows = M[c * BLK:(c + 1) * BLK]
            for cc in range(NCORE):
                ms[:BLK, cc * SPAD: cc * SPAD + BLK] = rows[:, cc * BLK:(cc + 1) * BLK]
                ms[BLK:, cc * SPAD: cc * SPAD + BLK] = 1.0
            return ms

        maskkl = np.ones((P, NT), np.float32)
        maskkl[BLK - 5 * P:, 5] = 0.0       # user strip 5: rows 110.. invalid
        maskkl[BLK - 5 * P:, 11] = 0.0      # item strip 5
        maskpr = np.zeros((P, 2), np.float32)
        maskpr[:RC - 97 * P, 0] = 1.0       # pairs col 97: p < 84 valid
        maskpr[RC - 97 * P:, 1] = 1.0e9

        x0_own = x0_full[c * SL:(c + 1) * SL].copy()

        in_maps.append({
            "x0_full": x0_full,
            "x0_own": x0_own,
            "att": att,
            "idx_g": idx_g,
            "idx_pru": _wrap16(pru),
            "idx_pri": _wrap16(pri),
            "mm_s": padmat(Mm),
            "nm_s": padmat(Nm),
            "maskkl": maskkl,
            "maskpr": maskpr,
        })
    return in_maps


def _assemble(results):
    user = np.concatenate([r["user_out"] for r in results], 0)
    item = np.concatenate([r["item_out"] for r in results], 0)
    pr = np.concatenate([
        r["pr_blob"].reshape(P, PRCH, PRCT).transpose(1, 2, 0).reshape(-1)[:RC]
        for r in results])
    pos_sum = np.float32(sum(np.float64(r["partials"][0, 0]) for r in results))
    klu = np.float32(sum(np.float64(r["partials"][1, 0]) for r in results))
    kln = np.float32(sum(np.float64(r["partials"][2, 0]) for r in results))
    loss = np.float32(pos_sum / np.float32(RTOT)
                      + np.float32(0.01) * (klu + kln))
    return user, item, loss, pr


# ----------------------------------------------------------------------------
# device program (SPMD, identical on all 8 cores)
# ----------------------------------------------------------------------------

def _build(nc):
    import concourse.tile as tile
    from concourse import mybir
    import concourse.bass as bass
    from concourse.masks import make_identity
    from contextlib import ExitStack

    f32 = mybir.dt.float32
    i16 = mybir.dt.int16
    AF = mybir.ActivationFunctionType
    OP = mybir.AluOpType
    AX = mybir.AxisListType

    # --- I/O -----------------------------------------------------------------
    x0_full = nc.declare_dram_parameter("x0_full", [TAB, D], f32, isOutput=False)
    x0_own_p = nc.declare_dram_parameter("x0_own", [SL, D], f32, isOutput=False)
    att_p = nc.declare_dram_parameter("att", [2 * D, 1], f32, isOutput=False)
    idx_g_p = nc.declare_dram_parameter("idx_g", [NT, P, NB * P // 16], i16, isOutput=False)
    idx_pru_p = nc.declare_dram_parameter("idx_pru", [P, RPAD // 16], i16, isOutput=False)
    idx_pri_p = nc.declare_dram_parameter("idx_pri", [P, RPAD // 16], i16, isOutput=False)
    mm_p = nc.declare_dram_parameter("mm_s", [SPAD, UW], f32, isOutput=False)
    nm_p = nc.declare_dram_parameter("nm_s", [SPAD, UW], f32, isOutput=False)
    maskkl_p = nc.declare_dram_parameter("maskkl", [P, NT], f32, isOutput=False)
    maskpr_p = nc.declare_dram_parameter("maskpr", [P, 2], f32, isOutput=False)

    user_out = nc.declare_dram_parameter("user_out", [BLK, D], f32, isOutput=True)
    item_out = nc.declare_dram_parameter("item_out", [BLK, D], f32, isOutput=True)
    pr_out = nc.declare_dram_parameter("pr_blob", [P, NPRT], f32, isOutput=True)
    partials_out = nc.declare_dram_parameter("partials", [3, 1], f32, isOutput=True)

    groups = [list(range(NCORE))]

    with tile.TileContext(nc) as tc, ExitStack() as ctx:
        dram = ctx.enter_context(tc.tile_pool(name="dram", bufs=1, space="DRAM"))
        consts = ctx.enter_context(tc.tile_pool(name="consts", bufs=1))
        scal = ctx.enter_context(tc.tile_pool(name="scal", bufs=1))
        persist = ctx.enter_context(tc.tile_pool(name="persist", bufs=1))
        bcastp = ctx.enter_context(tc.tile_pool(name="bcastp", bufs=1))

        # --- internal DRAM ---------------------------------------------------
        y1tab = dram.tile([TAB, TW], f32)
        y2pay = dram.tile([SL, TW], f32)
        y2tab = dram.tile([TAB, TW], f32, addr_space="Shared")
        x2pay = dram.tile([SL, D], f32)
        x2tab = dram.tile([TAB, D], f32, addr_space="Shared")

        # --- constants -------------------------------------------------------
        ident = consts.tile([P, P], f32)
        make_identity(nc, ident)
        ones128 = consts.tile([P, 1], f32)
        nc.vector.memset(ones128, 1.0)
        att2b = consts.tile([P, D], f32)
        nc.sync.dma_start(out=att2b, in_=bass.AP(
            tensor=att_p.ap().tensor, offset=D, ap=[[0, P], [1, D]]))
        maskklt = consts.tile([P, NT], f32)
        nc.sync.dma_start(out=maskklt, in_=maskkl_p[:, :])
        maskprt = consts.tile([P, 2], f32)
        nc.sync.dma_start(out=maskprt, in_=maskpr_p[:, :])
        idxg_all = consts.tile([P, NT, NB * P // 16], i16)
        nc.sync.dma_start(out=idxg_all, in_=idx_g_p[:].rearrange("t p s -> p t s"))
        idxpru = consts.tile([P, RPAD // 16], i16)
        nc.sync.dma_start(out=idxpru, in_=idx_pru_p[:, :])
        idxpri = consts.tile([P, RPAD // 16], i16)
        nc.sync.dma_start(out=idxpri, in_=idx_pri_p[:, :])

        X0own = persist.tile([P, NT, D], f32)
        nc.sync.dma_start(out=X0own, in_=x0_own_p[:].rearrange("(t p) d -> p t d", p=P))
        X1own = persist.tile([P, NT, D], f32)
        X2own = persist.tile([P, NT, D], f32)
        XTown = persist.tile([P, SL], f32)

        def rep16(t8):
            # [128, 8] -> broadcast-read AP [128, 8, 16]
            return bass.AP(tensor=t8.tensor, offset=t8.offset,
                           ap=[list(t8.ap[0]), list(t8.ap[1]), [0, 16]])

        # --- table build: per-node tables y = s*t(x)*x, s = exp(x.att2) -----
        def build_tables(src_tile, y_dst, s_dst, pool, psfx):
            t8 = pool.tile([P, K8], f32, tag="t8", name=f"t8{psfx}")
            nc.vector.tensor_reduce(
                out=t8, in_=src_tile.rearrange("p (k g) -> p k g", g=16),
                axis=AX.X, op=OP.add)
            qs = pool.tile([P, D], f32, tag="qscr", name=f"qscr{psfx}")
            qq = pool.tile([P, 1], f32, tag="qq", name=f"qq{psfx}")
            nc.vector.scalar_tensor_tensor(
                out=qs, in0=src_tile, scalar=1.0, in1=att2b,
                op0=OP.bypass, op1=OP.mult, accum_out=qq)
            nc.scalar.activation(s_dst, qq, AF.Exp)
            nc.vector.scalar_tensor_tensor(
                out=y_dst, in0=src_tile, scalar=s_dst, in1=rep16(t8),
                op0=OP.mult, op1=OP.mult)

        # Phase A: layer-1 tables for ALL nodes (replicated work, no comm)
        with tc.tile_pool(name="phA", bufs=1) as pA, \
             tc.tile_pool(name="phAs", bufs=3) as pAs:
            y_all = pA.tile([P, NBT, TW], f32)
            nc.vector.memset(y_all[:, :, D + 1:TW], 0.0)
            for bt in range(NBT):
                x0t = pAs.tile([P, D], f32, tag="x0t")
                nc.sync.dma_start(out=x0t, in_=x0_full[bt * P:(bt + 1) * P, :])
                build_tables(x0t, y_all[:, bt, 0:D], y_all[:, bt, D:D + 1],
                             pAs, "A")
            nc.sync.dma_start(out=y1tab[:].rearrange("(t p) w -> p t w", p=P),
                              in_=y_all)

        # --- conv layer ------------------------------------------------------
        def conv_layer(ytab_, xin, xout, pool, psfx):
            for t in range(NT):
                G = pool.tile([P, NB, TW], f32, tag="G", name=f"G{psfx}")
                nc.gpsimd.dma_gather(
                    out_ap=G[:], in_ap=ytab_[:, :], idxs_ap=idxg_all[:, t, :],
                    num_idxs=NB * P, num_idxs_reg=NB * P, elem_size=TW,
                    single_packet=False)
                agg = pool.tile([P, D], f32, tag="agg", name=f"agg{psfx}")
                nc.vector.tensor_reduce(
                    out=agg, in_=G[:, :, 0:D].rearrange("p m d -> p d m"),
                    axis=AX.X, op=OP.add)
                sigs = pool.tile([P, 1], f32, tag="sigs", name=f"sigs{psfx}")
                nc.vector.tensor_reduce(
                    out=sigs, in_=G[:, :, D:D + 1].rearrange("p m o -> p o m"),
                    axis=AX.X, op=OP.add)
                rsa = pool.tile([P, 1], f32, tag="rsa", name=f"rsa{psfx}")
                nc.vector.reciprocal(rsa, sigs)
                nc.vector.tensor_scalar(rsa, rsa, AA, None, op0=OP.mult)
                nc.vector.scalar_tensor_tensor(
                    out=xout[:, t, :], in0=agg, scalar=rsa, in1=xin[:, t, :],
                    op0=OP.mult, op1=OP.add)

        with tc.tile_pool(name="phB", bufs=2) as pB:
            conv_layer(y1tab, X0own, X1own, pB, "B")

        # Phase C: layer-2 table slice from own X1, exchange via AllGather
        with tc.tile_pool(name="phC", bufs=1) as pC, \
             tc.tile_pool(name="phCs", bufs=3) as pCs:
            y2_all = pC.tile([P, NT, TW], f32)
            nc.vector.memset(y2_all[:, :, D + 1:TW], 0.0)
            for t in range(NT):
                build_tables(X1own[:, t, :], y2_all[:, t, 0:D],
                             y2_all[:, t, D:D + 1], pCs, "C")
            nc.sync.dma_start(out=y2pay[:].rearrange("(t p) w -> p t w", p=P),
                              in_=y2_all)
        nc.gpsimd.collective_compute(
            "AllGather", mybir.AluOpType.bypass, replica_groups=groups,
            ins=[y2pay[:].rearrange("a b -> (a b)")],
            outs=[y2tab[:].rearrange("a b -> (a b)")])

        with tc.tile_pool(name="phD", bufs=2) as pD:
            conv_layer(y2tab, X1own, X2own, pD, "D")

        # zero the pad rows (tiles 5 and 11, partitions 110..127) so that
        # S pad columns are exactly 0 -> tanh 0, exp 1 (corrected by NPADC)
        nc.vector.tensor_scalar(X2own[:, 5, :], X2own[:, 5, :],
                                maskklt[:, 5:6], None, op0=OP.mult)
        nc.vector.tensor_scalar(X2own[:, 11, :], X2own[:, 11, :],
                                maskklt[:, 5:6], None, op0=OP.mult)

        # outputs: own slices
        nc.sync.dma_start(
            out=user_out[0:5 * P, :].rearrange("(t p) d -> p t d", p=P),
            in_=X2own[:, 0:5, :])
        nc.sync.dma_start(out=user_out[5 * P:BLK, :], in_=X2own[0:BLK - 5 * P, 5, :])
        nc.sync.dma_start(
            out=item_out[0:5 * P, :].rearrange("(t p) d -> p t d", p=P),
            in_=X2own[:, 6:11, :])
        nc.sync.dma_start(out=item_out[5 * P:BLK, :], in_=X2own[0:BLK - 5 * P, 11, :])

        # exchange full X2
        nc.sync.dma_start(out=x2pay[:].rearrange("(t p) d -> p t d", p=P),
                          in_=X2own)
        nc.gpsimd.collective_compute(
            "AllGather", mybir.AluOpType.bypass, replica_groups=groups,
            ins=[x2pay[:].rearrange("a b -> (a b)")],
            outs=[x2tab[:].rearrange("a b -> (a b)")])

        # Phase E: XT_all = X2^T in SBUF (PE transpose), XTown from X2own
        XT_all = bcastp.tile([P, TAB], f32, tag="sbc", name="XT_all")
        with tc.tile_pool(name="phE", bufs=3) as pE, \
             tc.tile_pool(name="phEp", bufs=4, space="PSUM") as pEp:
            for kt in range(NBT):
                x2t = pE.tile([P, D], f32, tag="x2t")
                nc.sync.dma_start(out=x2t, in_=x2tab[kt * P:(kt + 1) * P, :])
                pst = pEp.tile([P, P], f32, tag="pst")
                nc.tensor.transpose(out=pst, in_=x2t, identity=ident)
                nc.scalar.copy(out=XT_all[:, kt * P:(kt + 1) * P], in_=pst)
            for t in range(NT):
                pst2 = pEp.tile([P, P], f32, tag="pst")
                nc.tensor.transpose(out=pst2, in_=X2own[:, t, :], identity=ident)
                nc.scalar.copy(out=XTown[:, t * P:(t + 1) * P], in_=pst2)

        # Phase F: KL strips. 12 strips: 0..5 user (mm), 6..11 item (nm)
        sig_all = scal.tile([P, NT], f32)
        rsig_all = scal.tile([P, NT], f32)
        pv_all = scal.tile([P, NT], f32)
        pa_all = scal.tile([P, NT], f32)
        pvt_all = scal.tile([P, NT], f32)
        pt_all = scal.tile([P, NT], f32)
        sxr_all = scal.tile([P, NT], f32)

        with tc.tile_pool(name="phFm", bufs=2) as pFm, \
             tc.tile_pool(name="phFc", bufs=2) as pFc, \
             tc.tile_pool(name="phFs", bufs=2) as pFs, \
             tc.tile_pool(name="phFp", bufs=4, space="PSUM") as pFp:
            for st in range(2 * NSTRIP):
                item_side = st >= NSTRIP
                s = st - NSTRIP if item_side else st
                mpar = nm_p if item_side else mm_p
                mm = pFm.tile([P, UW], f32, tag="mm", name=f"mm{st}")
                nc.sync.dma_start(out=mm, in_=mpar[s * P:(s + 1) * P, :])
                sigp = pFs.tile([P, NCH], f32, tag="sigp", name=f"sigp{st}")
                for k in range(NCH):
                    nc.vector.tensor_reduce(
                        out=sigp[:, k:k + 1], in_=mm[:, k * CHW:(k + 1) * CHW],
                        axis=AX.X, op=OP.add)
                nc.vector.tensor_reduce(out=sig_all[:, st:st + 1], in_=sigp,
                                        axis=AX.X, op=OP.add)
                nc.vector.reciprocal(rsig_all[:, st:st + 1], sig_all[:, st:st + 1])

                lhsT = XTown[:, (SPAD if item_side else 0) + s * P:
                             (SPAD if item_side else 0) + (s + 1) * P]
                ptp = pFs.tile([P, NCH], f32, tag="ptp", name=f"ptp{st}")
                sxp = pFs.tile([P, NCH], f32, tag="sxp", name=f"sxp{st}")
                pvp = pFs.tile([P, NCH], f32, tag="pvp", name=f"pvp{st}")
                pap = pFs.tile([P, NCH], f32, tag="pap", name=f"pap{st}")
                pvtp = pFs.tile([P, NCH], f32, tag="pvtp", name=f"pvtp{st}")
                for k in range(NCH):
                    ps = pFp.tile([P, CHW], f32, tag="ps", name=f"ps{st}_{k}")
                    us = k * CHW
                    while us < (k + 1) * CHW:
                        cblk = us // SPAD
                        loc = us % SPAD
                        run = min((k + 1) * CHW - us, SPAD - loc)
                        tcol = cblk * SL + (SPAD if item_side else 0) + loc
                        nc.tensor.matmul(
                            out=ps[:, us - k * CHW: us - k * CHW + run],
                            lhsT=lhsT, rhs=XT_all[:, tcol:tcol + run],
                            start=True, stop=True)
                        us += run
                    tch = pFc.tile([P, CHW], f32, tag="tch", name=f"t{st}_{k}")
                    nc.scalar.activation(tch, ps, AF.Tanh, scale=0.5,
                                         accum_out=ptp[:, k:k + 1])
                    # exp(t/2): result only needed for its row-sum; write the
                    # elementwise output into the (dead) PSUM tile
                    nc.scalar.activation(ps, tch, AF.Exp, scale=0.5,
                                         accum_out=sxp[:, k:k + 1])
                    mmk = mm[:, k * CHW:(k + 1) * CHW]
                    eych = pFc.tile([P, CHW], f32, tag="eych", name=f"ey{st}_{k}")
                    nc.scalar.activation(eych, mmk, AF.Exp,
                                         scale=rsig_all[:, st:st + 1])
                    vch = pFc.tile([P, CHW], f32, tag="vch", name=f"v{st}_{k}")
                    nc.vector.tensor_scalar(vch, eych, 1.0, None, op0=OP.subtract,
                                            op1=OP.add, accum_out=pvp[:, k:k + 1])
                    scr2 = pFc.tile([P, CHW], f32, tag="sctt", name=f"s2{st}_{k}")
                    nc.vector.scalar_tensor_tensor(
                        out=scr2, in0=vch, scalar=1.0, in1=mmk,
                        op0=OP.bypass, op1=OP.mult, accum_out=pap[:, k:k + 1])
                    scr3 = pFc.tile([P, CHW], f32, tag="sctt", name=f"s3{st}_{k}")
                    nc.vector.scalar_tensor_tensor(
                        out=scr3, in0=vch, scalar=1.0, in1=tch,
                        op0=OP.bypass, op1=OP.mult, accum_out=pvtp[:, k:k + 1])
                nc.vector.tensor_reduce(out=pt_all[:, st:st + 1], in_=ptp,
                                        axis=AX.X, op=OP.add)
                nc.vector.tensor_reduce(out=sxr_all[:, st:st + 1], in_=sxp,
                                        axis=AX.X, op=OP.add)
                nc.vector.tensor_reduce(out=pv_all[:, st:st + 1], in_=pvp,
                                        axis=AX.X, op=OP.add)
                nc.vector.tensor_reduce(out=pa_all[:, st:st + 1], in_=pap,
                                        axis=AX.X, op=OP.add)
                nc.vector.tensor_reduce(out=pvt_all[:, st:st + 1], in_=pvtp,
                                        axis=AX.X, op=OP.add)

        # Phase G: pr dot products from gathered X2 rows
        pr_t = scal.tile([P, NPRT], f32)
        with tc.tile_pool(name="phG", bufs=2) as pG:
            for ci in range(PRCH):
                GU = pG.tile([P, PRCT, D], f32, tag="GU", name=f"GU{ci}")
                nc.gpsimd.dma_gather(
                    out_ap=GU[:], in_ap=x2tab[:, :],
                    idxs_ap=idxpru[:, ci * (PRCI // 16):(ci + 1) * (PRCI // 16)],
                    num_idxs=PRCI, num_idxs_reg=PRCI, elem_size=D,
                    single_packet=False)
                GV = pG.tile([P, PRCT, D], f32, tag="GV", name=f"GV{ci}")
                nc.gpsimd.dma_gather(
                    out_ap=GV[:], in_ap=x2tab[:, :],
                    idxs_ap=idxpri[:, ci * (PRCI // 16):(ci + 1) * (PRCI // 16)],
                    num_idxs=PRCI, num_idxs_reg=PRCI, elem_size=D,
                    single_packet=False)
                GP = pG.tile([P, PRCT, D], f32, tag="GP", name=f"GP{ci}")
                nc.vector.tensor_mul(GP, GU, GV)
                nc.vector.tensor_reduce(
                    out=pr_t[:, ci * PRCT:(ci + 1) * PRCT], in_=GP,
                    axis=AX.X, op=OP.add)
        nc.sync.dma_start(out=pr_out[:, :], in_=pr_t)

        # pads -> +inf-ish so softplus(-x) = 0
        nc.vector.scalar_tensor_tensor(
            out=pr_t[:, 97:98], in0=pr_t[:, 97:98], scalar=maskprt[:, 0:1],
            in1=maskprt[:, 1:2], op0=OP.mult, op1=OP.add)
        nc.vector.memset(pr_t[:, 98:NPRT], 1.0e9)

        # --- final combine ----------------------------------------------------
        SMa = scal.tile([P, NT], f32)
        nc.vector.tensor_scalar(SMa, pv_all, float(MLEN), None, op0=OP.add)
        rSMa = scal.tile([P, NT], f32)
        nc.vector.reciprocal(rSMa, SMa)
        sxr2 = scal.tile([P, NT], f32)
        nc.vector.tensor_scalar(sxr2, sxr_all, NPADC, None, op0=OP.subtract)
        ratio = scal.tile([P, NT], f32)
        nc.vector.tensor_mul(ratio, sxr2, rSMa)
        Lt = scal.tile([P, NT], f32)
        nc.scalar.activation(Lt, ratio, AF.Ln)
        a1 = scal.tile([P, NT], f32)
        nc.vector.tensor_mul(a1, pa_all, rsig_all)
        a2 = scal.tile([P, NT], f32)
        nc.vector.tensor_add(a2, pt_all, pvt_all)
        b2 = scal.tile([P, NT], f32)
        nc.vector.tensor_add(b2, a2, SMa)
        c2 = scal.tile([P, NT], f32)
        nc.vector.scalar_tensor_tensor(out=c2, in0=b2, scalar=-0.5, in1=a1,
                                       op0=OP.mult, op1=OP.add)
        nn_ = scal.tile([P, NT], f32)
        nc.vector.tensor_scalar(nn_, c2, 1.0, None, op0=OP.add)
        nr = scal.tile([P, NT], f32)
        nc.vector.tensor_mul(nr, nn_, rSMa)
        kl = scal.tile([P, NT], f32)
        nc.vector.scalar_tensor_tensor(out=kl, in0=nr, scalar=0.5, in1=Lt,
                                       op0=OP.add, op1=OP.add)
        klm = scal.tile([P, NT], f32)
        nc.vector.tensor_mul(klm, kl, maskklt)
        ku = scal.tile([P, 1], f32)
        nc.vector.tensor_reduce(out=ku, in_=klm[:, 0:NSTRIP], axis=AX.X, op=OP.add)
        kn = scal.tile([P, 1], f32)
        nc.vector.tensor_reduce(out=kn, in_=klm[:, NSTRIP:NT], axis=AX.X, op=OP.add)

        # positive-sum: softplus(-pr) = relu(-pr) + ln(1 + exp(-|pr|))
        posv = scal.tile([P, 1], f32)
        pr_ab = scal.tile([P, NPRT], f32)
        nc.scalar.activation(pr_ab, pr_t, AF.Abs)
        pr_e = scal.tile([P, NPRT], f32)
        nc.scalar.activation(pr_e, pr_ab, AF.Exp, scale=-1.0)
        pr_l = scal.tile([P, NPRT], f32)
        nc.scalar.activation(pr_l, pr_e, AF.Ln, bias=1.0)
        pr_z = scal.tile([P, NPRT], f32)
        nc.scalar.activation(pr_z, pr_t, AF.Relu, scale=-1.0)
        scrpr = scal.tile([P, NPRT], f32)
        nc.vector.tensor_add(scrpr, pr_l, pr_z)
        nc.vector.tensor_reduce(out=posv, in_=scrpr, axis=AX.X, op=OP.add)

        cat3 = scal.tile([P, 3], f32)
        nc.vector.tensor_copy(cat3[:, 0:1], posv)
        nc.vector.tensor_copy(cat3[:, 1:2], ku)
        nc.vector.tensor_copy(cat3[:, 2:3], kn)
        with tc.tile_pool(name="pp", bufs=1, space="PSUM") as pp:
            ps3 = pp.tile([3, 1], f32)
            nc.tensor.matmul(out=ps3, lhsT=cat3, rhs=ones128, start=True, stop=True)
            part_t = scal.tile([3, 1], f32)
            nc.vector.tensor_copy(part_t, ps3)
        nc.sync.dma_start(out=partials_out[:, :], in_=part_t)


# ----------------------------------------------------------------------------
# entry points
# ----------------------------------------------------------------------------

_CACHE = {}


def _get_nc():
    if "nc" not in _CACHE:
        from concourse import bacc
        nc = bacc.Bacc("TRN2", target_bir_lowering=False, debug=False,
                       num_devices=NCORE)
        _build(nc)
        nc.compile()
        _CACHE["nc"] = nc
    return _CACHE["nc"]


def _run(inputs, trace=False):
    from concourse.bass_utils import run_bass_kernel_spmd
    nc = _get_nc()
    in_maps = _prep(inputs)
    r = run_bass_kernel_spmd(nc, in_maps, core_ids=list(range(NCORE)),
                             trace=trace)
    outs = _assemble(r.results)
    return outs, r


def kernel(**inputs):
    outs, _ = _run(inputs, trace=False)
    return outs


if __name__ == "__main__":
    # smoke-test the program build only
    nc = _get_nc()
    print("program built ok;",
          sum(len(getattr(f, 'instructions', [])) for f in nc.m.functions),
          "instructions")
